# revision 9
# baseline (speedup 1.0000x reference)
"""Cross-modality attention Trainium2 kernel (8 NeuronCores, SPMD).

Problem: 3 modalities feat0..2 of [B=2, C=256, H=W=48]. For each modality i:
  ctx_i = sum_j softmax(Q_i K_j^T / sqrt(C)) V_j   (9 full NxN attentions, N=2304)
  out_i = Conv1x1(gate_i * ctx_i + (1-gate_i) * feat_i),  gate_i = sigmoid(Conv1x1(feat_i))

Sharding: core r handles batch b = r//4 and query-token slice s = r%4 (576 of the
2304 tokens) for all 3 modalities, flash-attention style with K/V replicated
(computed on-device from the full per-batch features).

Device algorithm per core (all matmuls bf16, fp32 PSUM accumulation):
- Q/K projections are folded into one matrix on the host:
  scores^T = X_j^T (Wk^T Wq / sqrt(C)) X_i = X_j^T Qg_i, so only one projection
  (Qg) per modality and the K projection/storage disappears entirely.
- Scores are computed TRANSPOSED ([key, query] layout) directly by matmul, so no
  transpose of the probability matrix is ever needed.  exp() without
  max-subtraction (scores are bounded: |s| < ~2 for this problem's distribution).
- V^T is computed directly in [token, channel] layout with a ones-column
  appended; the attention row-sum (softmax denominator) then falls out of the
  same matmul that computes P @ V, and normalization is a per-partition
  scalar multiply fused with the cross-modality accumulation.
"""

import os
from contextlib import ExitStack

import ml_dtypes
import numpy as np

import concourse.bass as bass
import concourse.tile as tile
from concourse import bacc, mybir
from concourse.bass_utils import run_bass_kernel_spmd
from concourse.masks import make_identity

B, C, H, W = 2, 256, 48, 48
N = H * W            # 2304 tokens
NCORES = 8
NSLICE = 4           # query slices per batch
QS = N // NSLICE     # 576 query tokens per core
KC = N // 128        # 18 key chunks of 128
QH = QS // 2         # 288: scores free-dim half (one PSUM bank)
MQ = (QS + 127) // 128  # 5 query m-chunks for the ctx matmul (last is 64)
XCH = N // QS        # 4 x-load chunks of 576 per half

F32 = mybir.dt.float32
BF16 = mybir.dt.bfloat16
AF = mybir.ActivationFunctionType
ALU = mybir.AluOpType


def _emit(ctx: ExitStack, tc: tile.TileContext, io: dict):
    nc = tc.nc

    # ---- pools ----------------------------------------------------------
    P = 128
    persist = ctx.enter_context(tc.tile_pool(name="persist", bufs=1))
    stgx = ctx.enter_context(tc.tile_pool(name="stgx", bufs=6))
    stgq = ctx.enter_context(tc.tile_pool(name="stgq", bufs=2))
    pt_pool = ctx.enter_context(tc.tile_pool(name="pt", bufs=2))
    ctx_pool = ctx.enter_context(tc.tile_pool(name="ctxp", bufs=2))
    ctxt_pool = ctx.enter_context(tc.tile_pool(name="ctxt", bufs=2))
    fus_pool = ctx.enter_context(tc.tile_pool(name="fus", bufs=2))
    tmpq_pool = ctx.enter_context(tc.tile_pool(name="tmpq", bufs=2))
    rp_pool = ctx.enter_context(tc.tile_pool(name="rp", bufs=6))
    osb_pool = ctx.enter_context(tc.tile_pool(name="osb", bufs=3))
    ps_s = ctx.enter_context(tc.tile_pool(name="ps_s", bufs=2, space="PSUM"))
    ps_c = ctx.enter_context(tc.tile_pool(name="ps_c", bufs=2, space="PSUM"))
    ps_w = ctx.enter_context(tc.tile_pool(name="ps_w", bufs=2, space="PSUM"))

    # ---- constants / weights (host pre-casts weights to bf16) -----------
    pos_sb = persist.tile([P, 2, 1], F32, tag="pos")
    gb_sb = persist.tile([P, 2, 1], F32, tag="gb")
    vb3_sb = persist.tile([P, 2, 1], F32, tag="vb3")
    outb_sb = persist.tile([P, 2, 1], F32, tag="outb")
    for nm, t_ in (("pos", pos_sb), ("gate_b", gb_sb), ("v_b3", vb3_sb),
                   ("out_b", outb_sb)):
        nc.sync.dma_start(out=t_[:, :, 0], in_=io[nm].rearrange("(t p) -> p t", p=P))

    wnames = ["m_t", "v_wt", "gate_wt", "out_wt"]
    wbf = {}
    for wn in wnames:
        wt = persist.tile([P, 2, C], BF16, tag=f"w_{wn}", name=f"w_{wn}")
        nc.sync.dma_start(out=wt[:, :, :],
                          in_=io[wn].rearrange("(t p) d -> p t d", p=P))
        wbf[wn] = wt

    xbf = [persist.tile([P, 2, N], BF16, tag=f"xbf{j}", name=f"xbf{j}")
           for j in range(3)]
    xqbf = [persist.tile([P, 2, QS], BF16, tag=f"xqbf{i}", name=f"xqbf{i}")
            for i in range(3)]
    vst = [persist.tile([P, KC, 257], BF16, tag=f"vst{j}", name=f"vst{j}")
           for j in range(3)]
    qg = [persist.tile([P, 2, QS], BF16, tag=f"qg{i}", name=f"qg{i}")
          for i in range(3)]
    gate = [persist.tile([P, 2, QS], BF16, tag=f"gate{i}", name=f"gate{i}")
            for i in range(3)]

    def load_xq(i):
        stg = stgq.tile([P, 2, QS], F32, tag="stgq")
        nc.sync.dma_start(out=stg[:, :, :],
                          in_=io[f"xq{i}"].rearrange("(t p) q -> p t q", p=P))
        for t in range(2):
            nc.scalar.activation(out=xqbf[i][:, t, :], in_=stg[:, t, :],
                                 func=AF.Identity, bias=pos_sb[:, t, :],
                                 scale=1.0)

    def load_x(j):
        # chunked [128, 576] loads so downstream compute starts early
        for t in range(2):
            for xc in range(XCH):
                stg = stgx.tile([P, QS], F32, tag="stgx")
                nc.gpsimd.dma_start(
                    out=stg[:, :],
                    in_=io[f"x{j}"][t * P:(t + 1) * P, xc * QS:(xc + 1) * QS])
                nc.vector.tensor_scalar_add(xbf[j][:, t, xc * QS:(xc + 1) * QS],
                                            stg[:, :], pos_sb[:, t, :])

    def project_v(j):
        # V^T[n, c] = sum_cin X[cin, n] * v_w[c, cin]  -> [128-token chunks, 256]
        for kc in range(KC):
            pv = ps_w.tile([P, 512], F32, tag="ps_w")
            for t in range(2):
                nc.tensor.matmul(pv[:, 0:C], xbf[j][:, t, kc * P:(kc + 1) * P],
                                 wbf["v_wt"][:, t, :], start=(t == 0), stop=(t == 1))
            nc.vector.tensor_copy(vst[j][:, kc, 0:C], pv[:, 0:C])
        nc.vector.memset(vst[j][:, :, 256:257], 1.0)

    def project_qg(i):
        # Qg = (Wk^T Wq / sqrt(C)) @ X_i, query slice only. [c_out, QS]
        for u in range(2):
            for qh in range(2):
                pq = ps_w.tile([P, 512], F32, tag="ps_w")
                for t in range(2):
                    nc.tensor.matmul(
                        pq[:, 0:QH], wbf["m_t"][:, t, u * P:(u + 1) * P],
                        xqbf[i][:, t, qh * QH:(qh + 1) * QH],
                        start=(t == 0), stop=(t == 1))
                nc.vector.tensor_copy(qg[i][:, u, qh * QH:(qh + 1) * QH], pq[:, 0:QH])

    def project_gate(i):
        # gate = sigmoid(Wg X + bg) on ScalarE (the sigmoid table set loads
        # before the first exp since all sigmoids are emitted first)
        for u in range(2):
            for qh in range(2):
                pg = ps_w.tile([P, 512], F32, tag="ps_w")
                for t in range(2):
                    nc.tensor.matmul(
                        pg[:, 0:QH], wbf["gate_wt"][:, t, u * P:(u + 1) * P],
                        xqbf[i][:, t, qh * QH:(qh + 1) * QH],
                        start=(t == 0), stop=(t == 1))
                nc.scalar.activation(out=gate[i][:, u, qh * QH:(qh + 1) * QH],
                                     in_=pg[:, 0:QH], func=AF.Sigmoid,
                                     bias=gb_sb[:, u, :], scale=1.0)

    def attention_pair(i, j, ctx_sb):
        # scores^T = X_j^T @ Qg_i, per 128-key chunk; exp -> P^T (bf16);
        # ctx_aug = P^T.T @ [V^T | 1]; normalize+accumulate into ctx_sb.
        pt = pt_pool.tile([P, KC, QS], BF16, tag="pt")
        for kc in range(KC):
            ps = ps_s.tile([P, 2, 512], F32, tag="ps_s")
            for t in range(2):
                for qh in range(2):
                    nc.tensor.matmul(
                        ps[:, qh, 0:QH], xbf[j][:, t, kc * P:(kc + 1) * P],
                        qg[i][:, t, qh * QH:(qh + 1) * QH],
                        start=(t == 0), stop=(t == 1), skip_group_check=True)
            nc.scalar.activation(
                out=pt[:, kc, :].rearrange("p (a b) -> p a b", a=2),
                in_=ps[:, :, 0:QH], func=AF.Exp)
        for mq in range(MQ):
            mm = min(P, QS - mq * P)
            pc = ps_c.tile([P, 512], F32, tag="ps_c")
            for kc in range(KC):
                nc.tensor.matmul(pc[0:mm, 0:257],
                                 pt[:, kc, mq * P:mq * P + mm],
                                 vst[j][:, kc, :],
                                 start=(kc == 0), stop=(kc == KC - 1))
            rp = rp_pool.tile([P, 1], F32, tag="rp")
            nc.vector.reciprocal(rp[0:mm, :], pc[0:mm, 256:257])
            if j == 0:
                nc.vector.tensor_scalar_mul(ctx_sb[0:mm, mq, :], pc[0:mm, 0:C],
                                            rp[0:mm, :])
            else:
                nc.vector.scalar_tensor_tensor(
                    out=ctx_sb[0:mm, mq, :], in0=pc[0:mm, 0:C], scalar=rp[0:mm, :],
                    in1=ctx_sb[0:mm, mq, :], op0=ALU.mult, op1=ALU.add)

    def finish_modality(i, ctx_sb):
        # transpose ctx to [C, QS], add 3*v_b; gate-blend; output conv; DMA out.
        ctx_t = ctxt_pool.tile([P, 2, QS], F32, tag="ctxt")
        for u in range(2):
            for mq in range(MQ):
                mm = min(P, QS - mq * P)
                pt_ps = ps_w.tile([P, 512], F32, tag="ps_w")
                nc.tensor.transpose(pt_ps[:, 0:mm],
                                    ctx_sb[0:mm, mq, u * P:(u + 1) * P],
                                    ident[0:mm, 0:mm])
                nc.vector.tensor_scalar_add(ctx_t[:, u, mq * P:mq * P + mm],
                                            pt_ps[:, 0:mm], vb3_sb[:, u, :])
        fus = fus_pool.tile([P, 2, QS], BF16, tag="fus")
        for u in range(2):
            diff = tmpq_pool.tile([P, QS], F32, tag="tmpq")
            nc.vector.tensor_sub(diff[:, :], ctx_t[:, u, :], xqbf[i][:, u, :])
            nc.vector.tensor_mul(diff[:, :], diff[:, :], gate[i][:, u, :])
            nc.vector.tensor_add(fus[:, u, :], diff[:, :], xqbf[i][:, u, :])
        for u in range(2):
            for qh in range(2):
                po = ps_w.tile([P, 512], F32, tag="ps_w")
                for t in range(2):
                    nc.tensor.matmul(
                        po[:, 0:QH], wbf["out_wt"][:, t, u * P:(u + 1) * P],
                        fus[:, t, qh * QH:(qh + 1) * QH],
                        start=(t == 0), stop=(t == 1))
                osb = osb_pool.tile([P, QH], F32, tag="osb")
                nc.scalar.activation(out=osb[:, :], in_=po[:, 0:QH],
                                     func=AF.Identity, bias=outb_sb[:, u, :],
                                     scale=1.0)
                nc.sync.dma_start(
                    out=io["out"][i, u * P:(u + 1) * P, qh * QH:(qh + 1) * QH],
                    in_=osb[:, :])

    # ---- schedule -------------------------------------------------------
    for i in range(3):
        load_xq(i)
    for i in range(3):
        project_gate(i)
        project_qg(i)
    load_x(0)
    ident = persist.tile([P, P], F32, tag="ident")
    make_identity(nc, ident)

    project_v(0)
    ctx_sbs = {}
    for i in range(3):
        ctx_sbs[i] = ctx_pool.tile([P, MQ, C], F32, tag="ctxp", name=f"ctx{i}")

    # modality-major pair order; V_j for j>0 is produced while attention on
    # earlier pairs keeps the PE busy.
    load_x(1)
    attention_pair(0, 0, ctx_sbs[0])
    project_v(1)
    load_x(2)
    attention_pair(0, 1, ctx_sbs[0])
    project_v(2)
    attention_pair(0, 2, ctx_sbs[0])
    finish_modality(0, ctx_sbs[0])
    for i in (1, 2):
        for j in range(3):
            attention_pair(i, j, ctx_sbs[i])
        finish_modality(i, ctx_sbs[i])


def _build():
    nc = bacc.Bacc("TRN2", target_bir_lowering=False, debug=False,
                   num_devices=NCORES)
    io = {}
    for j in range(3):
        io[f"x{j}"] = nc.declare_dram_parameter(f"x{j}", [C, N], F32, isOutput=False)
        io[f"xq{j}"] = nc.declare_dram_parameter(f"xq{j}", [C, QS], F32, isOutput=False)
    for wn in ["m_t", "v_wt", "gate_wt", "out_wt"]:
        io[wn] = nc.declare_dram_parameter(wn, [C, C], BF16, isOutput=False)
    for vn in ["pos", "gate_b", "v_b3", "out_b"]:
        io[vn] = nc.declare_dram_parameter(vn, [C], F32, isOutput=False)
    io["out"] = nc.declare_dram_parameter("out", [3, C, QS], F32, isOutput=True)

    with tile.TileContext(nc) as tc:
        with ExitStack() as ctx:
            _emit(ctx, tc, io)
    nc.compile()
    return nc


_CACHED_NC = None


def _get_nc():
    global _CACHED_NC
    if _CACHED_NC is None:
        _CACHED_NC = _build()
    return _CACHED_NC


def _run(inputs: dict, trace: bool = False, tmpdir: str | None = None):
    f32 = np.float32
    bf16 = ml_dtypes.bfloat16
    feats = [np.ascontiguousarray(inputs[f"feat{j}"], dtype=f32).reshape(B, C, N)
             for j in range(3)]
    q_w = np.asarray(inputs["q_w"], f32)
    k_w = np.asarray(inputs["k_w"], f32)
    for bn in ("q_b", "k_b"):
        if not np.all(np.asarray(inputs[bn]) == 0):
            raise NotImplementedError(f"{bn} != 0 unsupported (spec fill=zeros)")
    scale = np.float32(C ** -0.5)
    m_t = np.ascontiguousarray(((q_w.T @ k_w) * scale).astype(bf16))
    v_wt = np.ascontiguousarray(np.asarray(inputs["v_w"], f32).T.astype(bf16))
    gate_wt = np.ascontiguousarray(np.asarray(inputs["gate_w"], f32).T.astype(bf16))
    out_wt = np.ascontiguousarray(np.asarray(inputs["out_w"], f32).T.astype(bf16))
    pos = np.ascontiguousarray(np.asarray(inputs["pos_embedding"], f32).reshape(C))
    gate_b = np.ascontiguousarray(np.asarray(inputs["gate_b"], f32).reshape(C))
    v_b3 = np.ascontiguousarray(3.0 * np.asarray(inputs["v_b"], f32).reshape(C))
    out_b = np.ascontiguousarray(np.asarray(inputs["out_b"], f32).reshape(C))

    shared = {"m_t": m_t, "v_wt": v_wt, "gate_wt": gate_wt, "out_wt": out_wt,
              "pos": pos, "gate_b": gate_b, "v_b3": v_b3, "out_b": out_b}
    in_maps = []
    for r in range(NCORES):
        b, s = r // NSLICE, r % NSLICE
        im = dict(shared)
        for j in range(3):
            im[f"x{j}"] = np.ascontiguousarray(feats[j][b])
            im[f"xq{j}"] = np.ascontiguousarray(
                feats[j][b][:, s * QS:(s + 1) * QS])
        in_maps.append(im)

    nc = _get_nc()
    res = run_bass_kernel_spmd(nc, in_maps, core_ids=list(range(NCORES)),
                               trace=trace, tmpdir=tmpdir)
    full = np.empty((3, B, C, N), dtype=f32)
    for r in range(NCORES):
        b, s = r // NSLICE, r % NSLICE
        full[:, b, :, s * QS:(s + 1) * QS] = res.results[r]["out"]
    full = full.reshape(3, B, C, H, W)
    return (full[0], full[1], full[2]), res


def kernel(**inputs):
    outs, _ = _run(inputs, trace=bool(os.environ.get("KERNEL_TRACE")))
    return outs


# revision 10
# speedup vs baseline: 1.0327x; 1.0327x over previous
"""Cross-modality attention Trainium2 kernel (8 NeuronCores, SPMD).

Problem: 3 modalities feat0..2 of [B=2, C=256, H=W=48]. For each modality i:
  ctx_i = sum_j softmax(Q_i K_j^T / sqrt(C)) V_j   (9 full NxN attentions, N=2304)
  out_i = Conv1x1(gate_i * ctx_i + (1-gate_i) * feat_i),  gate_i = sigmoid(Conv1x1(feat_i))

Sharding: core r handles batch b = r//4 and query-token slice s = r%4 (576 of the
2304 tokens) for all 3 modalities, flash-attention style with K/V replicated
(computed on-device from the full per-batch features).

Device algorithm per core (all matmuls bf16, fp32 PSUM accumulation):
- Q/K projections are folded into one matrix on the host:
  scores^T = X_j^T (Wk^T Wq / sqrt(C)) X_i = X_j^T Qg_i, so only one projection
  (Qg) per modality and the K projection/storage disappears entirely.
- Scores are computed TRANSPOSED ([key, query] layout) directly by matmul, so no
  transpose of the probability matrix is ever needed.  exp() without
  max-subtraction (scores are bounded: |s| < ~2 for this problem's distribution).
- V^T is computed directly in [token, channel] layout with a ones-column
  appended; the attention row-sum (softmax denominator) then falls out of the
  same matmul that computes P @ V, and normalization is a per-partition
  scalar multiply fused with the cross-modality accumulation.
"""

import os
from contextlib import ExitStack

import ml_dtypes
import numpy as np

import concourse.bass as bass
import concourse.tile as tile
from concourse import bacc, mybir
from concourse.bass_utils import run_bass_kernel_spmd
from concourse.masks import make_identity

B, C, H, W = 2, 256, 48, 48
N = H * W            # 2304 tokens
NCORES = 8
NSLICE = 4           # query slices per batch
QS = N // NSLICE     # 576 query tokens per core
KC = N // 128        # 18 key chunks of 128
QH = QS // 2         # 288: scores free-dim half (one PSUM bank)
MQ = (QS + 127) // 128  # 5 query m-chunks for the ctx matmul (last is 64)
XCH = N // QS        # 4 x-load chunks of 576 per half

F32 = mybir.dt.float32
BF16 = mybir.dt.bfloat16
AF = mybir.ActivationFunctionType
ALU = mybir.AluOpType


def _emit(ctx: ExitStack, tc: tile.TileContext, io: dict):
    nc = tc.nc

    # ---- pools ----------------------------------------------------------
    P = 128
    persist = ctx.enter_context(tc.tile_pool(name="persist", bufs=1))
    stgx = ctx.enter_context(tc.tile_pool(name="stgx", bufs=6))
    stgq = ctx.enter_context(tc.tile_pool(name="stgq", bufs=2))
    pt_pool = ctx.enter_context(tc.tile_pool(name="pt", bufs=2))
    ctx_pool = ctx.enter_context(tc.tile_pool(name="ctxp", bufs=2))
    ctxt_pool = ctx.enter_context(tc.tile_pool(name="ctxt", bufs=2))
    fus_pool = ctx.enter_context(tc.tile_pool(name="fus", bufs=2))
    tmpq_pool = ctx.enter_context(tc.tile_pool(name="tmpq", bufs=2))
    rp_pool = ctx.enter_context(tc.tile_pool(name="rp", bufs=6))
    osb_pool = ctx.enter_context(tc.tile_pool(name="osb", bufs=3))
    ps_s = ctx.enter_context(tc.tile_pool(name="ps_s", bufs=2, space="PSUM"))
    ps_c = ctx.enter_context(tc.tile_pool(name="ps_c", bufs=2, space="PSUM"))
    ps_w = ctx.enter_context(tc.tile_pool(name="ps_w", bufs=2, space="PSUM"))

    # ---- constants / weights (host pre-casts weights to bf16) -----------
    pos_sb = persist.tile([P, 2, 1], F32, tag="pos")
    gb_sb = persist.tile([P, 2, 1], F32, tag="gb")
    vb3_sb = persist.tile([P, 2, 1], F32, tag="vb3")
    outb_sb = persist.tile([P, 2, 1], F32, tag="outb")
    nc.sync.dma_start(out=pos_sb[:, :, 0],
                      in_=io["pos"].rearrange("(t p) -> p t", p=P))

    wnames = ["m_t", "v_wt", "gate_wt", "out_wt"]
    wbf = {}
    for wn in wnames:
        wbf[wn] = persist.tile([P, 2, C], BF16, tag=f"w_{wn}", name=f"w_{wn}")

    def load_weights_and_consts():
        for wn in wnames:
            nc.sync.dma_start(out=wbf[wn][:, :, :],
                              in_=io[wn].rearrange("(t p) d -> p t d", p=P))
        for nm, t_ in (("gate_b", gb_sb), ("v_b3", vb3_sb), ("out_b", outb_sb)):
            nc.sync.dma_start(out=t_[:, :, 0],
                              in_=io[nm].rearrange("(t p) -> p t", p=P))

    xbf = [persist.tile([P, 2, N], BF16, tag=f"xbf{j}", name=f"xbf{j}")
           for j in range(3)]
    xqbf = [persist.tile([P, 2, QS], BF16, tag=f"xqbf{i}", name=f"xqbf{i}")
            for i in range(3)]
    vst = [persist.tile([P, KC, 257], BF16, tag=f"vst{j}", name=f"vst{j}")
           for j in range(3)]
    qg = [persist.tile([P, 2, QS], BF16, tag=f"qg{i}", name=f"qg{i}")
          for i in range(3)]
    gate = [persist.tile([P, 2, QS], BF16, tag=f"gate{i}", name=f"gate{i}")
            for i in range(3)]

    def load_xq(i):
        for t in range(2):
            stg = stgq.tile([P, QS], F32, tag="stgq")
            nc.sync.dma_start(out=stg[:, :],
                              in_=io[f"xq{i}"][t * P:(t + 1) * P, :])
            nc.scalar.activation(out=xqbf[i][:, t, :], in_=stg[:, :],
                                 func=AF.Identity, bias=pos_sb[:, t, :],
                                 scale=1.0)

    def load_x(j):
        # chunked [128, 576] loads so downstream compute starts early
        for t in range(2):
            for xc in range(XCH):
                stg = stgx.tile([P, QS], F32, tag="stgx")
                nc.gpsimd.dma_start(
                    out=stg[:, :],
                    in_=io[f"x{j}"][t * P:(t + 1) * P, xc * QS:(xc + 1) * QS])
                nc.vector.tensor_scalar_add(xbf[j][:, t, xc * QS:(xc + 1) * QS],
                                            stg[:, :], pos_sb[:, t, :])

    def project_v(j):
        # V^T[n, c] = sum_cin X[cin, n] * v_w[c, cin]  -> [128-token chunks, 256]
        for kc in range(KC):
            pv = ps_w.tile([P, 512], F32, tag="ps_w")
            for t in range(2):
                nc.tensor.matmul(pv[:, 0:C], xbf[j][:, t, kc * P:(kc + 1) * P],
                                 wbf["v_wt"][:, t, :], start=(t == 0), stop=(t == 1))
            nc.vector.tensor_copy(vst[j][:, kc, 0:C], pv[:, 0:C])
        nc.vector.memset(vst[j][:, :, 256:257], 1.0)

    def project_qg(i):
        # Qg = (Wk^T Wq / sqrt(C)) @ X_i, query slice only. [c_out, QS]
        for u in range(2):
            for qh in range(2):
                pq = ps_w.tile([P, 512], F32, tag="ps_w")
                for t in range(2):
                    nc.tensor.matmul(
                        pq[:, 0:QH], wbf["m_t"][:, t, u * P:(u + 1) * P],
                        xqbf[i][:, t, qh * QH:(qh + 1) * QH],
                        start=(t == 0), stop=(t == 1))
                nc.vector.tensor_copy(qg[i][:, u, qh * QH:(qh + 1) * QH], pq[:, 0:QH])

    def project_gate(i):
        # gate = sigmoid(Wg X + bg) on ScalarE (the sigmoid table set loads
        # before the first exp since all sigmoids are emitted first)
        for u in range(2):
            for qh in range(2):
                pg = ps_w.tile([P, 512], F32, tag="ps_w")
                for t in range(2):
                    nc.tensor.matmul(
                        pg[:, 0:QH], wbf["gate_wt"][:, t, u * P:(u + 1) * P],
                        xqbf[i][:, t, qh * QH:(qh + 1) * QH],
                        start=(t == 0), stop=(t == 1))
                nc.scalar.activation(out=gate[i][:, u, qh * QH:(qh + 1) * QH],
                                     in_=pg[:, 0:QH], func=AF.Sigmoid,
                                     bias=gb_sb[:, u, :], scale=1.0)

    def attention_pair(i, j, ctx_sb):
        # scores^T = X_j^T @ Qg_i, per 128-key chunk; exp -> P^T (bf16);
        # ctx_aug = P^T.T @ [V^T | 1]; normalize+accumulate into ctx_sb.
        pt = pt_pool.tile([P, KC, QS], BF16, tag="pt")
        for kc in range(KC):
            ps = ps_s.tile([P, 2, 512], F32, tag="ps_s")
            for t in range(2):
                for qh in range(2):
                    nc.tensor.matmul(
                        ps[:, qh, 0:QH], xbf[j][:, t, kc * P:(kc + 1) * P],
                        qg[i][:, t, qh * QH:(qh + 1) * QH],
                        start=(t == 0), stop=(t == 1), skip_group_check=True)
            nc.scalar.activation(
                out=pt[:, kc, :].rearrange("p (a b) -> p a b", a=2),
                in_=ps[:, :, 0:QH], func=AF.Exp)
        for mq in range(MQ):
            mm = min(P, QS - mq * P)
            pc = ps_c.tile([P, 512], F32, tag="ps_c")
            for kc in range(KC):
                nc.tensor.matmul(pc[0:mm, 0:257],
                                 pt[:, kc, mq * P:mq * P + mm],
                                 vst[j][:, kc, :],
                                 start=(kc == 0), stop=(kc == KC - 1))
            rp = rp_pool.tile([P, 1], F32, tag="rp")
            nc.vector.reciprocal(rp[0:mm, :], pc[0:mm, 256:257])
            if j == 0:
                nc.vector.tensor_scalar_mul(ctx_sb[0:mm, mq, :], pc[0:mm, 0:C],
                                            rp[0:mm, :])
            else:
                nc.vector.scalar_tensor_tensor(
                    out=ctx_sb[0:mm, mq, :], in0=pc[0:mm, 0:C], scalar=rp[0:mm, :],
                    in1=ctx_sb[0:mm, mq, :], op0=ALU.mult, op1=ALU.add)

    def finish_modality(i, ctx_sb):
        # transpose ctx to [C, QS], add 3*v_b; gate-blend; output conv; DMA out.
        ctx_t = ctxt_pool.tile([P, 2, QS], F32, tag="ctxt")
        for u in range(2):
            for mq in range(MQ):
                mm = min(P, QS - mq * P)
                pt_ps = ps_w.tile([P, 512], F32, tag="ps_w")
                nc.tensor.transpose(pt_ps[:, 0:mm],
                                    ctx_sb[0:mm, mq, u * P:(u + 1) * P],
                                    ident[0:mm, 0:mm])
                nc.vector.tensor_scalar_add(ctx_t[:, u, mq * P:mq * P + mm],
                                            pt_ps[:, 0:mm], vb3_sb[:, u, :])
        fus = fus_pool.tile([P, 2, QS], BF16, tag="fus")
        for u in range(2):
            diff = tmpq_pool.tile([P, QS], F32, tag="tmpq")
            nc.vector.tensor_sub(diff[:, :], ctx_t[:, u, :], xqbf[i][:, u, :])
            nc.vector.tensor_mul(diff[:, :], diff[:, :], gate[i][:, u, :])
            nc.vector.tensor_add(fus[:, u, :], diff[:, :], xqbf[i][:, u, :])
        for u in range(2):
            for qh in range(2):
                po = ps_w.tile([P, 512], F32, tag="ps_w")
                for t in range(2):
                    nc.tensor.matmul(
                        po[:, 0:QH], wbf["out_wt"][:, t, u * P:(u + 1) * P],
                        fus[:, t, qh * QH:(qh + 1) * QH],
                        start=(t == 0), stop=(t == 1))
                osb = osb_pool.tile([P, QH], F32, tag="osb")
                nc.scalar.activation(out=osb[:, :], in_=po[:, 0:QH],
                                     func=AF.Identity, bias=outb_sb[:, u, :],
                                     scale=1.0)
                nc.sync.dma_start(
                    out=io["out"][i, u * P:(u + 1) * P, qh * QH:(qh + 1) * QH],
                    in_=osb[:, :])

    # ---- schedule -------------------------------------------------------
    for i in range(3):
        load_xq(i)
    load_weights_and_consts()
    for i in range(3):
        project_gate(i)
        project_qg(i)
    load_x(0)
    ident = persist.tile([P, P], F32, tag="ident")
    make_identity(nc, ident)

    project_v(0)
    ctx_sbs = {}
    for i in range(3):
        ctx_sbs[i] = ctx_pool.tile([P, MQ, C], F32, tag="ctxp", name=f"ctx{i}")

    # modality-major pair order; V_j for j>0 is produced while attention on
    # earlier pairs keeps the PE busy.
    attention_pair(0, 0, ctx_sbs[0])
    load_x(1)
    project_v(1)
    attention_pair(0, 1, ctx_sbs[0])
    load_x(2)
    project_v(2)
    attention_pair(0, 2, ctx_sbs[0])
    finish_modality(0, ctx_sbs[0])
    for i in (1, 2):
        for j in range(3):
            attention_pair(i, j, ctx_sbs[i])
        finish_modality(i, ctx_sbs[i])


def _build():
    nc = bacc.Bacc("TRN2", target_bir_lowering=False, debug=False,
                   num_devices=NCORES)
    io = {}
    for j in range(3):
        io[f"x{j}"] = nc.declare_dram_parameter(f"x{j}", [C, N], F32, isOutput=False)
        io[f"xq{j}"] = nc.declare_dram_parameter(f"xq{j}", [C, QS], F32, isOutput=False)
    for wn in ["m_t", "v_wt", "gate_wt", "out_wt"]:
        io[wn] = nc.declare_dram_parameter(wn, [C, C], BF16, isOutput=False)
    for vn in ["pos", "gate_b", "v_b3", "out_b"]:
        io[vn] = nc.declare_dram_parameter(vn, [C], F32, isOutput=False)
    io["out"] = nc.declare_dram_parameter("out", [3, C, QS], F32, isOutput=True)

    with tile.TileContext(nc) as tc:
        with ExitStack() as ctx:
            _emit(ctx, tc, io)
    nc.compile()
    return nc


_CACHED_NC = None


def _get_nc():
    global _CACHED_NC
    if _CACHED_NC is None:
        _CACHED_NC = _build()
    return _CACHED_NC


def _run(inputs: dict, trace: bool = False, tmpdir: str | None = None):
    f32 = np.float32
    bf16 = ml_dtypes.bfloat16
    feats = [np.ascontiguousarray(inputs[f"feat{j}"], dtype=f32).reshape(B, C, N)
             for j in range(3)]
    q_w = np.asarray(inputs["q_w"], f32)
    k_w = np.asarray(inputs["k_w"], f32)
    for bn in ("q_b", "k_b"):
        if not np.all(np.asarray(inputs[bn]) == 0):
            raise NotImplementedError(f"{bn} != 0 unsupported (spec fill=zeros)")
    scale = np.float32(C ** -0.5)
    m_t = np.ascontiguousarray(((q_w.T @ k_w) * scale).astype(bf16))
    v_wt = np.ascontiguousarray(np.asarray(inputs["v_w"], f32).T.astype(bf16))
    gate_wt = np.ascontiguousarray(np.asarray(inputs["gate_w"], f32).T.astype(bf16))
    out_wt = np.ascontiguousarray(np.asarray(inputs["out_w"], f32).T.astype(bf16))
    pos = np.ascontiguousarray(np.asarray(inputs["pos_embedding"], f32).reshape(C))
    gate_b = np.ascontiguousarray(np.asarray(inputs["gate_b"], f32).reshape(C))
    v_b3 = np.ascontiguousarray(3.0 * np.asarray(inputs["v_b"], f32).reshape(C))
    out_b = np.ascontiguousarray(np.asarray(inputs["out_b"], f32).reshape(C))

    shared = {"m_t": m_t, "v_wt": v_wt, "gate_wt": gate_wt, "out_wt": out_wt,
              "pos": pos, "gate_b": gate_b, "v_b3": v_b3, "out_b": out_b}
    in_maps = []
    for r in range(NCORES):
        b, s = r // NSLICE, r % NSLICE
        im = dict(shared)
        for j in range(3):
            im[f"x{j}"] = np.ascontiguousarray(feats[j][b])
            im[f"xq{j}"] = np.ascontiguousarray(
                feats[j][b][:, s * QS:(s + 1) * QS])
        in_maps.append(im)

    nc = _get_nc()
    res = run_bass_kernel_spmd(nc, in_maps, core_ids=list(range(NCORES)),
                               trace=trace, tmpdir=tmpdir)
    full = np.empty((3, B, C, N), dtype=f32)
    for r in range(NCORES):
        b, s = r // NSLICE, r % NSLICE
        full[:, b, :, s * QS:(s + 1) * QS] = res.results[r]["out"]
    full = full.reshape(3, B, C, H, W)
    return (full[0], full[1], full[2]), res


def kernel(**inputs):
    outs, _ = _run(inputs, trace=bool(os.environ.get("KERNEL_TRACE")))
    return outs
